# revision 18
# baseline (speedup 1.0000x reference)
"""Trainium2 Bass kernel for GQA attention layer (RoPE + causal + GQA 32q/8kv).

Self-contained: hardcodes shapes from the problem spec.
  hidden_states [2, 2048, 4096] f32, positions [2, 2048] i32,
  Wq [4096, 4096], Wk [1024, 4096], Wv [1024, 4096], Wo [4096, 4096]  (all f32)
Sharding: tensor-parallel over heads across 8 cores. Core c gets kv head c and
q heads 4c..4c+3. Each core computes its partial Wo output; host sums partials.

Per-core schedule (per batch, chunk = 512 tokens = one q-chunk):
  proj(0), attn(0), proj(1), wo(0), attn(1), ..., proj(3), wo(2), attn(3), wo(3)
Attention: scores [k, q] per (head, k-tile), exp'd probs become the stationary
operand of N=129 ctx matmuls against V with an appended ones column, which
yields the softmax denominator for free; per-subtile reciprocal + scale, then
PE-transpose into ctxT for the output projection.
"""

import math
import os
import sys
import types

import numpy as np
import ml_dtypes

BF16NP = ml_dtypes.bfloat16

# ---- problem constants (hardcoded per spec) ----
P = 128
B = 2
S = 2048            # tokens per batch
HID = 4096
NH, NKV, HD = 32, 8, 128
NCORES = 8
HPC = NH // NCORES  # q heads per core (4)
T = B * S
SCALE = 1.0 / math.sqrt(HD)
ROPE_BASE = 10000.0

QC = 512            # q-chunk == proj chunk (tokens)
NQC = S // QC       # 4
NST = QC // P       # q subtiles per chunk (4)
NKK = HID // P      # 32 contraction tiles
NKT = S // P        # 16 k tiles per batch
OCW = 512           # Wo output column chunk
HOC = HID // OCW    # 8

LAST = {}           # exec_time_ns etc from the most recent run


def _install_ntff_hook():
    """Register the axon NTFF profiling hook (image's antenv lacks axon_hooks)."""
    if "antenv.axon_hooks" in sys.modules:
        return
    try:
        import antenv
        mod = types.ModuleType("antenv.axon_hooks")
        _box = [None]
        mod.set_axon_ntff_profile_hook = lambda h: _box.__setitem__(0, h)
        mod.get_axon_ntff_profile_hook = lambda: _box[0]
        sys.modules["antenv.axon_hooks"] = mod
        antenv.axon_hooks = mod
        from trn_agent_boot.trn_boot import _ntff_profile_via_ctypes
        mod.set_axon_ntff_profile_hook(
            _ntff_profile_via_ctypes("/opt/axon/libaxon_pjrt.so")
        )
    except Exception:
        pass


def build_graph():
    import concourse.bacc as bacc
    import concourse.mybir as mybir
    import concourse.tile as tile
    from contextlib import ExitStack

    BF = mybir.dt.bfloat16
    F32 = mybir.dt.float32
    Exp = mybir.ActivationFunctionType.Exp

    NKH = NKK // 2  # x half-tiles

    nc = bacc.Bacc(None)
    xT_h = nc.declare_dram_parameter("xT", [HID, T], BF, isOutput=False)
    wq_h = nc.declare_dram_parameter("wqT", [HID, HPC * HD], BF, isOutput=False)
    wk_h = nc.declare_dram_parameter("wkT", [HID, HD], BF, isOutput=False)
    wv_h = nc.declare_dram_parameter("wvT", [HID, HD], BF, isOutput=False)
    wo_h = nc.declare_dram_parameter("woT", [HPC * HD, HID], BF, isOutput=False)
    cos_h = nc.declare_dram_parameter("cos2", [P, T], BF, isOutput=False)
    sin_h = nc.declare_dram_parameter("sin2", [P, T], BF, isOutput=False)
    msk_h = nc.declare_dram_parameter("masks", [P, P], F32, isOutput=False)
    idn_h = nc.declare_dram_parameter("iden", [P, P], BF, isOutput=False)
    out_h = nc.declare_dram_parameter("out", [T, HID], BF, isOutput=True)

    xT_r = xT_h[:, :].rearrange("(ko ki) s -> ki ko s", ki=P)
    wq_r = wq_h[:, :].rearrange("(ko ki) d -> ki ko d", ki=P)
    wk_r = wk_h[:, :].rearrange("(ko ki) d -> ki ko d", ki=P)
    wv_r = wv_h[:, :].rearrange("(ko ki) d -> ki ko d", ki=P)
    wo_r = wo_h[:, :].rearrange("(oo oi) h -> oi oo h", oi=P)

    with tile.TileContext(nc) as tc, ExitStack() as ctx:
        wpool = ctx.enter_context(tc.tile_pool(name="wpool", bufs=1))
        xpool = ctx.enter_context(tc.tile_pool(name="xpool", bufs=8))
        cspool = ctx.enter_context(tc.tile_pool(name="cspool", bufs=2))
        qkvpool = ctx.enter_context(tc.tile_pool(name="qkvpool", bufs=1))
        vtpool = ctx.enter_context(tc.tile_pool(name="vtpool", bufs=2))
        rpool = ctx.enter_context(tc.tile_pool(name="rpool", bufs=2))
        pbpool = ctx.enter_context(tc.tile_pool(name="pbpool", bufs=3))
        scpool = ctx.enter_context(tc.tile_pool(name="scpool", bufs=4))
        cnpool = ctx.enter_context(tc.tile_pool(name="cnpool", bufs=3))
        obpool = ctx.enter_context(tc.tile_pool(name="obpool", bufs=2))

        psS = ctx.enter_context(tc.tile_pool(name="psS", bufs=2, space="PSUM"))
        psX = ctx.enter_context(tc.tile_pool(name="psX", bufs=6, space="PSUM"))

        # --- persistent weights / tables ---
        wq_sb = wpool.tile([P, NKK, HPC * HD], BF)
        for wi in range(4):
            lo, hi = wi * NKK // 4, (wi + 1) * NKK // 4
            nc.scalar.dma_start(out=wq_sb[:, lo:hi, :], in_=wq_r[:, lo:hi, :])
        wk_sb = wpool.tile([P, NKK, HD], BF)
        nc.scalar.dma_start(out=wk_sb, in_=wk_r)
        wv_sb = wpool.tile([P, NKK, HD], BF)
        nc.scalar.dma_start(out=wv_sb, in_=wv_r)
        wo_sb = wpool.tile([P, HPC, HID], BF)
        mneg_sb = wpool.tile([P, P], F32)
        nc.scalar.dma_start(out=mneg_sb, in_=msk_h[:, :])
        iden_sb = wpool.tile([P, P], BF)
        nc.scalar.dma_start(out=iden_sb, in_=idn_h[:, :])

        def rope(ps, dst, cs, sn):
            """Neox RoPE on [128 d, n] tile: rows 0:64 = first half of head dim."""
            qf = rpool.tile([P, QC], BF, tag="qf")
            nc.any.tensor_copy(out=qf, in_=ps)
            qs = rpool.tile([P, QC], BF, tag="qs")
            nc.gpsimd.dma_start(out=qs[0:64, :], in_=qf[64:128, :])
            nc.gpsimd.dma_start(out=qs[64:128, :], in_=qf[0:64, :])
            nc.any.tensor_mul(out=qf, in0=qf, in1=cs)
            nc.any.tensor_mul(out=qs, in0=qs, in1=sn)
            nc.any.tensor_add(out=dst, in0=qf, in1=qs)

        def emit_x(b, t):
            """Prefetch cos/sin + x for chunk (b, t) in quarter tiles."""
            c0 = b * S + t * QC
            c1 = c0 + QC
            cs = cspool.tile([P, QC], BF, tag="cos", name="cs")
            nc.sync.dma_start(out=cs, in_=cos_h[:, c0:c1])
            sn = cspool.tile([P, QC], BF, tag="sin", name="sn")
            nc.sync.dma_start(out=sn, in_=sin_h[:, c0:c1])
            xq = []
            for qtr in range(4):
                xt_ = xpool.tile([P, NKK // 4, QC], BF, tag="x", name="xt_")
                nc.sync.dma_start(
                    out=xt_,
                    in_=xT_r[:, qtr * (NKK // 4):(qtr + 1) * (NKK // 4), c0:c1],
                )
                xq.append(xt_)
            return cs, sn, xq

        def proj_chunk(b, t, qT, kT, v, pre):
            cs, sn, xq = pre

            def xt(kk):
                return xq[kk // (NKK // 4)][:, kk % (NKK // 4), :]

            for g in range(HPC):
                ps = psX.tile([P, QC], F32, tag="px")
                for kk in range(NKK):
                    nc.tensor.matmul(
                        ps,
                        lhsT=wq_sb[:, kk, g * HD:(g + 1) * HD],
                        rhs=xt(kk),
                        start=(kk == 0),
                        stop=(kk == NKK - 1),
                    )
                rope(ps, qT[:, g, t * QC:(t + 1) * QC], cs, sn)
            ps = psX.tile([P, QC], F32, tag="px")
            for kk in range(NKK):
                nc.tensor.matmul(
                    ps, lhsT=wk_sb[:, kk, :], rhs=xt(kk),
                    start=(kk == 0), stop=(kk == NKK - 1),
                )
            rope(ps, kT[:, t * QC:(t + 1) * QC], cs, sn)
            # V in vT orientation (N=512 matmuls), then DMA-transpose to [s, d]
            pv = psX.tile([P, QC], F32, tag="px")
            for kk in range(NKK):
                nc.tensor.matmul(
                    pv, lhsT=wv_sb[:, kk, :], rhs=xt(kk),
                    start=(kk == 0), stop=(kk == NKK - 1),
                )
            vt = vtpool.tile([P, QC], BF, tag="vt")
            nc.any.tensor_copy(out=vt, in_=pv)
            for ss in range(NST):
                # DMA-transpose needs a contiguous [P, P] destination; stage
                # then copy into the strided v slice on the gpsimd queue.
                vstg = vtpool.tile([P, P], BF, tag="vstg")
                nc.scalar.dma_start(
                    out=vstg,
                    in_=vt[:, ss * P:(ss + 1) * P],
                    transpose=True,
                )
                nc.gpsimd.dma_start(out=v[:, t * NST + ss, 0:P], in_=vstg)

        def attn_chunk(b, qc, qT, kT, v, ctxT):
            nkt = NST * (qc + 1)
            for h in range(HPC):
                pcs = []
                for _ in range(NST):
                    pc = psX.tile([P, 132], F32, tag="px", name="pc")
                    pcs.append(pc)
                for kt in range(nkt):
                    d = kt - NST * qc
                    c0 = d * P if d > 0 else 0
                    pss = psS.tile([P, QC], F32, tag="s")
                    nc.tensor.matmul(
                        pss[:, c0:QC],
                        lhsT=kT[:, kt * P:(kt + 1) * P],
                        rhs=qT[:, h, qc * QC + c0:(qc + 1) * QC],
                        start=True, stop=True,
                    )
                    if d >= 0:
                        nc.vector.tensor_add(
                            out=pss[:, d * P:(d + 1) * P],
                            in0=pss[:, d * P:(d + 1) * P],
                            in1=mneg_sb,
                        )
                    pb = pbpool.tile([P, QC], BF, tag="pb")
                    nc.scalar.activation(
                        out=pb[:, c0:QC], in_=pss[:, c0:QC], func=Exp,
                        scale=SCALE,
                    )
                    for st in range(NST):
                        qsi = qc * NST + st
                        if kt <= qsi:
                            nc.tensor.matmul(
                                pcs[st][:, 0:129],
                                lhsT=pb[:, st * P:(st + 1) * P],
                                rhs=v[:, kt, 0:129],
                                start=(kt == 0), stop=(kt == qsi),
                            )
                rcs = []
                for st in range(NST):
                    rc = scpool.tile([P, 1], F32)
                    nc.vector.reciprocal(out=rc, in_=pcs[st][:, 128:129])
                    rcs.append(rc)
                for st in range(NST):
                    qsi = qc * NST + st
                    cn = cnpool.tile([P, P], BF)
                    if st % 2 == 0:
                        nc.scalar.mul(out=cn, in_=pcs[st][:, 0:P], mul=rcs[st])
                    else:
                        nc.vector.tensor_scalar_mul(
                            out=cn, in0=pcs[st][:, 0:P], scalar1=rcs[st]
                        )
                    pt = psX.tile([P, P], BF, tag="px", name="pt")
                    nc.tensor.transpose(pt, cn, iden_sb)
                    if st % 2 == 0:
                        nc.vector.tensor_copy(
                            out=ctxT[:, h, qsi * P:(qsi + 1) * P], in_=pt
                        )
                    else:
                        nc.scalar.copy(
                            out=ctxT[:, h, qsi * P:(qsi + 1) * P], in_=pt
                        )

        def wo_block(b, qc, ctxT):
            for hc in range(HOC):
                for sg in range(2):
                    ob = obpool.tile([P, 2, OCW], BF, tag="ob")
                    for si2 in range(2):
                        si = sg * 2 + si2
                        po = psX.tile([P, OCW], F32, tag="px", name="po")
                        for ot in range(HPC):
                            q0 = qc * QC + si * P
                            nc.tensor.matmul(
                                po,
                                lhsT=ctxT[:, ot, q0:q0 + P],
                                rhs=wo_sb[:, ot, hc * OCW:(hc + 1) * OCW],
                                start=(ot == 0), stop=(ot == HPC - 1),
                            )
                        if (si2 + hc) % 2 == 0:
                            nc.vector.tensor_copy(out=ob[:, si2, :], in_=po)
                        else:
                            nc.scalar.copy(out=ob[:, si2, :], in_=po)
                    r0 = b * S + qc * QC + sg * 2 * P
                    orows = out_h[r0:r0 + 2 * P, hc * OCW:(hc + 1) * OCW]
                    nc.sync.dma_start(
                        out=orows.rearrange("(si p) h -> p si h", p=P), in_=ob
                    )

        nxt = emit_x(0, 0)
        for b in range(B):
            qT = qkvpool.tile([P, HPC, S], BF, tag="qT")
            kT = qkvpool.tile([P, S], BF, tag="kT")
            v = qkvpool.tile([P, NKT, 132], BF, tag="v")
            nc.vector.memset(v[:, :, 128:129], 1.0)
            ctxT = qkvpool.tile([P, HPC, S], BF, tag="ctxT")
            for t in range(NQC):
                cur = nxt
                if (b, t) != (B - 1, NQC - 1):
                    nxt = emit_x(b + (t + 1) // NQC, (t + 1) % NQC)
                proj_chunk(b, t, qT, kT, v, cur)
                if b == 0 and t == 0:
                    for wi in range(4):
                        lo, hi = wi * HID // 4, (wi + 1) * HID // 4
                        nc.scalar.dma_start(
                            out=wo_sb[:, :, lo:hi], in_=wo_r[:, :, lo:hi]
                        )
                if t >= 1:
                    wo_block(b, t - 1, ctxT)
                attn_chunk(b, t, qT, kT, v, ctxT)
            wo_block(b, NQC - 1, ctxT)

    nc.compile()
    return nc


_CACHE = {}


def _get_graph():
    if "nc" not in _CACHE:
        _CACHE["nc"] = build_graph()
    return _CACHE["nc"]


def _host_prep(hidden_states, positions, Wq, Wk, Wv, Wo):
    """Transpose/cast/slice inputs per core. Returns list of 8 input dicts."""
    x2 = np.ascontiguousarray(hidden_states.reshape(T, HID).T).astype(BF16NP)

    pos = positions.astype(np.float32)                      # [B, S]
    half = HD // 2
    inv_freq = 1.0 / (ROPE_BASE ** (np.arange(half, dtype=np.float32) / half))
    ang = pos[:, :, None] * inv_freq[None, None, :]         # [B, S, 64]
    cos = np.cos(ang)
    sin = np.sin(ang)
    cosT = np.concatenate([cos[b].T for b in range(B)], axis=1)   # [64, T]
    sinT = np.concatenate([sin[b].T for b in range(B)], axis=1)
    cos2 = np.concatenate([cosT, cosT], axis=0).astype(BF16NP)    # [128, T]
    sin2 = np.concatenate([-sinT, sinT], axis=0).astype(BF16NP)

    r = np.arange(P)
    masks = np.where(r[:, None] <= r[None, :], 0.0, -1e30).astype(np.float32)
    iden = np.eye(P, dtype=np.float32).astype(BF16NP)

    in_maps = []
    for c in range(NCORES):
        qs = slice(c * HPC * HD, (c + 1) * HPC * HD)
        ks = slice(c * HD, (c + 1) * HD)
        in_maps.append({
            "xT": x2,
            "wqT": np.ascontiguousarray(Wq[qs, :].T).astype(BF16NP),
            "wkT": np.ascontiguousarray(Wk[ks, :].T).astype(BF16NP),
            "wvT": np.ascontiguousarray(Wv[ks, :].T).astype(BF16NP),
            "woT": np.ascontiguousarray(Wo[:, qs].T).astype(BF16NP),
            "cos2": cos2,
            "sin2": sin2,
            "masks": masks,
            "iden": iden,
        })
    return in_maps


def kernel(hidden_states, positions, Wq, Wk, Wv, Wo):
    from concourse.bass_utils import run_bass_kernel_spmd

    trace = bool(os.environ.get("CLAUDE_KERNEL_TRACE"))
    if trace:
        _install_ntff_hook()

    nc = _get_graph()
    in_maps = _host_prep(
        np.asarray(hidden_states), np.asarray(positions),
        np.asarray(Wq), np.asarray(Wk), np.asarray(Wv), np.asarray(Wo),
    )
    res = run_bass_kernel_spmd(
        nc, in_maps, core_ids=list(range(NCORES)), trace=trace,
    )
    LAST["exec_time_ns"] = res.exec_time_ns
    LAST["profile_json"] = res.profile_json
    if res.instructions_and_trace is not None:
        LAST["trace_path"] = res.instructions_and_trace[1]

    acc = np.zeros((T, HID), np.float32)
    for c in range(NCORES):
        acc += res.results[c]["out"].astype(np.float32)
    return acc.reshape(B, S, HID)


# revision 21
# speedup vs baseline: 1.0571x; 1.0571x over previous
"""Trainium2 Bass kernel for GQA attention layer (RoPE + causal + GQA 32q/8kv).

Self-contained: hardcodes shapes from the problem spec.
  hidden_states [2, 2048, 4096] f32, positions [2, 2048] i32,
  Wq [4096, 4096], Wk [1024, 4096], Wv [1024, 4096], Wo [4096, 4096]  (all f32)
Sharding: tensor-parallel over heads across 8 cores. Core c gets kv head c and
q heads 4c..4c+3. Each core computes its partial Wo output; host sums partials.

Per-core schedule (per batch, chunk = 512 tokens = one q-chunk):
  proj(0), attn(0), proj(1), wo(0), attn(1), ..., proj(3), wo(2), attn(3), wo(3)
Attention: scores [k, q] per (head, k-tile), exp'd probs become the stationary
operand of N=129 ctx matmuls against V with an appended ones column, which
yields the softmax denominator for free; per-subtile reciprocal + scale, then
PE-transpose into ctxT for the output projection.
"""

import math
import os
import sys
import types

import numpy as np
import ml_dtypes

BF16NP = ml_dtypes.bfloat16

# ---- problem constants (hardcoded per spec) ----
P = 128
B = 2
S = 2048            # tokens per batch
HID = 4096
NH, NKV, HD = 32, 8, 128
NCORES = 8
HPC = NH // NCORES  # q heads per core (4)
T = B * S
SCALE = 1.0 / math.sqrt(HD)
ROPE_BASE = 10000.0

QC = 512            # q-chunk == proj chunk (tokens)
NQC = S // QC       # 4
NST = QC // P       # q subtiles per chunk (4)
NKK = HID // P      # 32 contraction tiles
NKT = S // P        # 16 k tiles per batch
OCW = 512           # Wo output column chunk
HOC = HID // OCW    # 8

LAST = {}           # exec_time_ns etc from the most recent run


def _install_ntff_hook():
    """Register the axon NTFF profiling hook (image's antenv lacks axon_hooks)."""
    if "antenv.axon_hooks" in sys.modules:
        return
    try:
        import antenv
        mod = types.ModuleType("antenv.axon_hooks")
        _box = [None]
        mod.set_axon_ntff_profile_hook = lambda h: _box.__setitem__(0, h)
        mod.get_axon_ntff_profile_hook = lambda: _box[0]
        sys.modules["antenv.axon_hooks"] = mod
        antenv.axon_hooks = mod
        from trn_agent_boot.trn_boot import _ntff_profile_via_ctypes
        mod.set_axon_ntff_profile_hook(
            _ntff_profile_via_ctypes("/opt/axon/libaxon_pjrt.so")
        )
    except Exception:
        pass


def build_graph():
    import concourse.bacc as bacc
    import concourse.mybir as mybir
    import concourse.tile as tile
    from contextlib import ExitStack

    BF = mybir.dt.bfloat16
    F32 = mybir.dt.float32
    Exp = mybir.ActivationFunctionType.Exp

    NKH = NKK // 2  # x half-tiles

    nc = bacc.Bacc(None)
    xT_h = nc.declare_dram_parameter("xT", [HID, T], BF, isOutput=False)
    wq_h = nc.declare_dram_parameter("wqT", [HID, HPC * HD], BF, isOutput=False)
    wk_h = nc.declare_dram_parameter("wkT", [HID, HD], BF, isOutput=False)
    wv_h = nc.declare_dram_parameter("wvT", [HID, HD], BF, isOutput=False)
    wo_h = nc.declare_dram_parameter("woT", [HPC * HD, HID], BF, isOutput=False)
    cos_h = nc.declare_dram_parameter("cos2", [P, T], BF, isOutput=False)
    sin_h = nc.declare_dram_parameter("sin2", [P, T], BF, isOutput=False)
    msk_h = nc.declare_dram_parameter("masks", [P, P], F32, isOutput=False)
    idn_h = nc.declare_dram_parameter("iden", [P, P], BF, isOutput=False)
    out_h = nc.declare_dram_parameter("out", [T, HID], BF, isOutput=True)

    xT_r = xT_h[:, :].rearrange("(ko ki) s -> ki ko s", ki=P)
    wq_r = wq_h[:, :].rearrange("(ko ki) d -> ki ko d", ki=P)
    wk_r = wk_h[:, :].rearrange("(ko ki) d -> ki ko d", ki=P)
    wv_r = wv_h[:, :].rearrange("(ko ki) d -> ki ko d", ki=P)
    wo_r = wo_h[:, :].rearrange("(oo oi) h -> oi oo h", oi=P)

    with tile.TileContext(nc) as tc, ExitStack() as ctx:
        wpool = ctx.enter_context(tc.tile_pool(name="wpool", bufs=1))
        xpool = ctx.enter_context(tc.tile_pool(name="xpool", bufs=6))
        cspool = ctx.enter_context(tc.tile_pool(name="cspool", bufs=2))
        qkvpool = ctx.enter_context(tc.tile_pool(name="qkvpool", bufs=1))
        vtpool = ctx.enter_context(tc.tile_pool(name="vtpool", bufs=2))
        rpool = ctx.enter_context(tc.tile_pool(name="rpool", bufs=2))
        pbpool = ctx.enter_context(tc.tile_pool(name="pbpool", bufs=3))
        scpool = ctx.enter_context(tc.tile_pool(name="scpool", bufs=4))
        cnpool = ctx.enter_context(tc.tile_pool(name="cnpool", bufs=3))
        obpool = ctx.enter_context(tc.tile_pool(name="obpool", bufs=2))
        ctpool = ctx.enter_context(tc.tile_pool(name="ctpool", bufs=2))

        psS = ctx.enter_context(tc.tile_pool(name="psS", bufs=2, space="PSUM"))
        psX = ctx.enter_context(tc.tile_pool(name="psX", bufs=6, space="PSUM"))

        # --- persistent weights / tables ---
        wq_sb = wpool.tile([P, NKK, HPC * HD], BF)
        for wi in range(4):
            lo, hi = wi * NKK // 4, (wi + 1) * NKK // 4
            nc.scalar.dma_start(out=wq_sb[:, lo:hi, :], in_=wq_r[:, lo:hi, :])
        wk_sb = wpool.tile([P, NKK, HD], BF)
        nc.scalar.dma_start(out=wk_sb, in_=wk_r)
        wv_sb = wpool.tile([P, NKK, HD], BF)
        nc.scalar.dma_start(out=wv_sb, in_=wv_r)
        wo_sb = wpool.tile([P, HPC, HID], BF)
        mneg_sb = wpool.tile([P, P], F32)
        nc.scalar.dma_start(out=mneg_sb, in_=msk_h[:, :])
        iden_sb = wpool.tile([P, P], BF)
        nc.scalar.dma_start(out=iden_sb, in_=idn_h[:, :])

        def rope(ps, dst, cs, sn):
            """Neox RoPE on [128 d, n] tile: rows 0:64 = first half of head dim."""
            qf = rpool.tile([P, QC], BF, tag="qf")
            nc.any.tensor_copy(out=qf, in_=ps)
            qs = rpool.tile([P, QC], BF, tag="qs")
            nc.gpsimd.dma_start(out=qs[0:64, :], in_=qf[64:128, :])
            nc.gpsimd.dma_start(out=qs[64:128, :], in_=qf[0:64, :])
            nc.any.tensor_mul(out=qf, in0=qf, in1=cs)
            nc.any.tensor_mul(out=qs, in0=qs, in1=sn)
            nc.any.tensor_add(out=dst, in0=qf, in1=qs)

        def emit_x(b, t):
            """Prefetch cos/sin + x for chunk (b, t) in quarter tiles."""
            c0 = b * S + t * QC
            c1 = c0 + QC
            cs = cspool.tile([P, QC], BF, tag="cos", name="cs")
            nc.sync.dma_start(out=cs, in_=cos_h[:, c0:c1])
            sn = cspool.tile([P, QC], BF, tag="sin", name="sn")
            nc.sync.dma_start(out=sn, in_=sin_h[:, c0:c1])
            xq = []
            for qtr in range(4):
                xt_ = xpool.tile([P, NKK // 4, QC], BF, tag="x", name="xt_")
                nc.sync.dma_start(
                    out=xt_,
                    in_=xT_r[:, qtr * (NKK // 4):(qtr + 1) * (NKK // 4), c0:c1],
                )
                xq.append(xt_)
            return cs, sn, xq

        def proj_chunk(b, t, qT, kT, v, pre):
            cs, sn, xq = pre

            def xt(kk):
                return xq[kk // (NKK // 4)][:, kk % (NKK // 4), :]

            for g in range(HPC):
                ps = psX.tile([P, QC], F32, tag="px")
                for kk in range(NKK):
                    nc.tensor.matmul(
                        ps,
                        lhsT=wq_sb[:, kk, g * HD:(g + 1) * HD],
                        rhs=xt(kk),
                        start=(kk == 0),
                        stop=(kk == NKK - 1),
                    )
                rope(ps, qT[:, g, t * QC:(t + 1) * QC], cs, sn)
            ps = psX.tile([P, QC], F32, tag="px")
            for kk in range(NKK):
                nc.tensor.matmul(
                    ps, lhsT=wk_sb[:, kk, :], rhs=xt(kk),
                    start=(kk == 0), stop=(kk == NKK - 1),
                )
            rope(ps, kT[:, t * QC:(t + 1) * QC], cs, sn)
            # V in vT orientation (N=512 matmuls), then DMA-transpose to [s, d]
            pv = psX.tile([P, QC], F32, tag="px")
            for kk in range(NKK):
                nc.tensor.matmul(
                    pv, lhsT=wv_sb[:, kk, :], rhs=xt(kk),
                    start=(kk == 0), stop=(kk == NKK - 1),
                )
            vt = vtpool.tile([P, QC], BF, tag="vt")
            nc.any.tensor_copy(out=vt, in_=pv)
            for ss in range(NST):
                # DMA-transpose needs a contiguous [P, P] destination; stage
                # then copy into the strided v slice on the gpsimd queue.
                vstg = vtpool.tile([P, P], BF, tag="vstg")
                nc.sync.dma_start(
                    out=vstg,
                    in_=vt[:, ss * P:(ss + 1) * P],
                    transpose=True,
                )
                nc.gpsimd.dma_start(out=v[:, t * NST + ss, 0:P], in_=vstg)

        def attn_chunk(b, qc, qT, kT, v, ctxT, filler=None):
            nkt = NST * (qc + 1)
            for h in range(HPC):
                if filler is not None:
                    filler(h)
                pcs = []
                for _ in range(NST):
                    pc = psX.tile([P, 132], F32, tag="px", name="pc")
                    pcs.append(pc)
                for kt in range(nkt):
                    d = kt - NST * qc
                    c0 = d * P if d > 0 else 0
                    pss = psS.tile([P, QC], F32, tag="s")
                    nc.tensor.matmul(
                        pss[:, c0:QC],
                        lhsT=kT[:, kt * P:(kt + 1) * P],
                        rhs=qT[:, h, qc * QC + c0:(qc + 1) * QC],
                        start=True, stop=True,
                    )
                    if d >= 0:
                        nc.vector.tensor_add(
                            out=pss[:, d * P:(d + 1) * P],
                            in0=pss[:, d * P:(d + 1) * P],
                            in1=mneg_sb,
                        )
                    pb = pbpool.tile([P, QC], BF, tag="pb")
                    nc.scalar.activation(
                        out=pb[:, c0:QC], in_=pss[:, c0:QC], func=Exp,
                        scale=SCALE,
                    )
                    for st in range(NST):
                        qsi = qc * NST + st
                        if kt <= qsi:
                            nc.tensor.matmul(
                                pcs[st][:, 0:129],
                                lhsT=pb[:, st * P:(st + 1) * P],
                                rhs=v[:, kt, 0:129],
                                start=(kt == 0), stop=(kt == qsi),
                            )
                rcs = []
                for st in range(NST):
                    rc = scpool.tile([P, 1], F32)
                    nc.vector.reciprocal(out=rc, in_=pcs[st][:, 128:129])
                    rcs.append(rc)
                for st in range(NST):
                    qsi = qc * NST + st
                    cn = cnpool.tile([P, P], BF)
                    if st % 2 == 0:
                        nc.scalar.mul(out=cn, in_=pcs[st][:, 0:P], mul=rcs[st])
                    else:
                        nc.vector.tensor_scalar_mul(
                            out=cn, in0=pcs[st][:, 0:P], scalar1=rcs[st]
                        )
                    pt = psX.tile([P, P], BF, tag="px", name="pt")
                    nc.tensor.transpose(pt, cn, iden_sb)
                    if st % 2 == 0:
                        nc.vector.tensor_copy(
                            out=ctxT[:, h, qsi * P:(qsi + 1) * P], in_=pt
                        )
                    else:
                        nc.scalar.copy(
                            out=ctxT[:, h, qsi * P:(qsi + 1) * P], in_=pt
                        )

        def wo_pair(b, qc, ctxT, hc0):
            for hc in (hc0, hc0 + 1):
                for sg in range(2):
                    ob = obpool.tile([P, 2, OCW], BF, tag="ob")
                    for si2 in range(2):
                        si = sg * 2 + si2
                        po = psX.tile([P, OCW], F32, tag="px", name="po")
                        for ot in range(HPC):
                            q0 = qc * QC + si * P
                            nc.tensor.matmul(
                                po,
                                lhsT=ctxT[:, ot, q0:q0 + P],
                                rhs=wo_sb[:, ot, hc * OCW:(hc + 1) * OCW],
                                start=(ot == 0), stop=(ot == HPC - 1),
                            )
                        if (si2 + hc) % 2 == 0:
                            nc.vector.tensor_copy(out=ob[:, si2, :], in_=po)
                        else:
                            nc.scalar.copy(out=ob[:, si2, :], in_=po)
                    r0 = b * S + qc * QC + sg * 2 * P
                    orows = out_h[r0:r0 + 2 * P, hc * OCW:(hc + 1) * OCW]
                    nc.sync.dma_start(
                        out=orows.rearrange("(si p) h -> p si h", p=P), in_=ob
                    )

        def wo_block(b, qc, ctxT):
            for hc0 in range(0, HOC, 2):
                wo_pair(b, qc, ctxT, hc0)

        nxt = emit_x(0, 0)
        for b in range(B):
            qT = qkvpool.tile([P, HPC, S], BF, tag="qT")
            kT = qkvpool.tile([P, S], BF, tag="kT")
            v = qkvpool.tile([P, NKT, 132], BF, tag="v")
            nc.vector.memset(v[:, :, 128:129], 1.0)
            ctxT = ctpool.tile([P, HPC, S], BF, tag="ctxT")
            for t in range(NQC):
                cur = nxt
                if (b, t) != (B - 1, NQC - 1):
                    nxt = emit_x(b + (t + 1) // NQC, (t + 1) % NQC)
                proj_chunk(b, t, qT, kT, v, cur)
                if b == 0 and t == 0:
                    for wi in range(4):
                        lo, hi = wi * HID // 4, (wi + 1) * HID // 4
                        nc.scalar.dma_start(
                            out=wo_sb[:, :, lo:hi], in_=wo_r[:, :, lo:hi]
                        )
                if t >= 1:
                    filler = (lambda h, _t=t: wo_pair(b, _t - 1, ctxT, 2 * h))
                elif b == 1:
                    filler = (lambda h: wo_pair(0, NQC - 1, ctxT0, 2 * h))
                else:
                    filler = None
                attn_chunk(b, t, qT, kT, v, ctxT, filler)
            if b == B - 1:
                wo_block(b, NQC - 1, ctxT)
            ctxT0 = ctxT

    nc.compile()
    return nc


_CACHE = {}


def _get_graph():
    if "nc" not in _CACHE:
        _CACHE["nc"] = build_graph()
    return _CACHE["nc"]


def _host_prep(hidden_states, positions, Wq, Wk, Wv, Wo):
    """Transpose/cast/slice inputs per core. Returns list of 8 input dicts."""
    x2 = np.ascontiguousarray(hidden_states.reshape(T, HID).T).astype(BF16NP)

    pos = positions.astype(np.float32)                      # [B, S]
    half = HD // 2
    inv_freq = 1.0 / (ROPE_BASE ** (np.arange(half, dtype=np.float32) / half))
    ang = pos[:, :, None] * inv_freq[None, None, :]         # [B, S, 64]
    cos = np.cos(ang)
    sin = np.sin(ang)
    cosT = np.concatenate([cos[b].T for b in range(B)], axis=1)   # [64, T]
    sinT = np.concatenate([sin[b].T for b in range(B)], axis=1)
    cos2 = np.concatenate([cosT, cosT], axis=0).astype(BF16NP)    # [128, T]
    sin2 = np.concatenate([-sinT, sinT], axis=0).astype(BF16NP)

    r = np.arange(P)
    masks = np.where(r[:, None] <= r[None, :], 0.0, -1e30).astype(np.float32)
    iden = np.eye(P, dtype=np.float32).astype(BF16NP)

    in_maps = []
    for c in range(NCORES):
        qs = slice(c * HPC * HD, (c + 1) * HPC * HD)
        ks = slice(c * HD, (c + 1) * HD)
        in_maps.append({
            "xT": x2,
            "wqT": np.ascontiguousarray(Wq[qs, :].T).astype(BF16NP),
            "wkT": np.ascontiguousarray(Wk[ks, :].T).astype(BF16NP),
            "wvT": np.ascontiguousarray(Wv[ks, :].T).astype(BF16NP),
            "woT": np.ascontiguousarray(Wo[:, qs].T).astype(BF16NP),
            "cos2": cos2,
            "sin2": sin2,
            "masks": masks,
            "iden": iden,
        })
    return in_maps


def kernel(hidden_states, positions, Wq, Wk, Wv, Wo):
    from concourse.bass_utils import run_bass_kernel_spmd

    trace = bool(os.environ.get("CLAUDE_KERNEL_TRACE"))
    if trace:
        _install_ntff_hook()

    nc = _get_graph()
    in_maps = _host_prep(
        np.asarray(hidden_states), np.asarray(positions),
        np.asarray(Wq), np.asarray(Wk), np.asarray(Wv), np.asarray(Wo),
    )
    res = run_bass_kernel_spmd(
        nc, in_maps, core_ids=list(range(NCORES)), trace=trace,
    )
    LAST["exec_time_ns"] = res.exec_time_ns
    LAST["profile_json"] = res.profile_json
    if res.instructions_and_trace is not None:
        LAST["trace_path"] = res.instructions_and_trace[1]

    acc = np.zeros((T, HID), np.float32)
    for c in range(NCORES):
        acc += res.results[c]["out"].astype(np.float32)
    return acc.reshape(B, S, HID)
